# revision 1
# baseline (speedup 1.0000x reference)
"""Trainium2 Bass kernel for the CoLL co-occurrence layer.

Math (per image):
    scale = 8/(max(x)-min(x)+1e-8)   (global over the whole batch)
    u     = (x - xmin)*scale ;  idx = clip(floor(u), 0, 7)
    y(p)  = sum_q w[q] * x(p+q) * co[idx_p, idx_q]   over 3x3 neighborhoods q

Reformulation used here (staircase basis):
    g_j(p) = 1[u(p) >= j]                    j = 0..7  (g_0 == 1)
    rho_i  = co[i, idx] = sum_j A[i,j] g_j   with A[i,j] = co[i,j]-co[i,j-1]
    v_i    = x * rho_i
    V_i    = conv3x3(v_i, w)                 (SAME, zero pad)
    y(p)   = V_{idx_p}(p)   -- selected via a chain of predicated copies on g_i

Mapping:
  - data-parallel: one image per NeuronCore (batch 8 over 8 cores); the global
    min/max is a 2-float AllReduce(max) of (-min, max).
  - layout [h=128 partitions, (w,c)=8192 free]; conv along h via banded-matrix
    matmuls on PE (contraction over h_in), conv along w via +-C free-dim reads
    of a zero-padded tile.
  - the 8x8 mix (rho from staircases) is a PE matmul kron(A[:,1:].T, I16) on
    partition-packed staircases (pack/unpack via small SBUF-SBUF DMAs); the
    g_0 column is folded into the PSUM-evacuation bias.
"""

from contextlib import ExitStack

import numpy as np

import concourse.bass as bass
import concourse.tile as tile
from concourse import mybir, bass_isa
from concourse.tile_rust import add_dep_helper

F32 = mybir.dt.float32
AX = mybir.AxisListType
OP = mybir.AluOpType

N, H, W, C = 8, 128, 128, 64
NB = 8
N_CORES = 8


def build_tables(co, w):
    """Host-side weight-matrix construction from the tiny co/w inputs (f32).

    The 8x8 staircase-mix is done per 32-row h-group in two output halves
    (i in 0..3 / 4..7), each accumulating two matmuls (staircases j=1..4 and
    j=5..7); the j=0 (ones) column is folded into a per-partition bias.
    SBUF partition starts are restricted to {0,32,64,96}, hence the 32-row
    granularity.
    """
    co = np.asarray(co, np.float32)
    w = np.asarray(w, np.float32)
    A = co - np.concatenate([np.zeros((NB, 1), np.float32), co[:, :-1]], axis=1)
    I32 = np.eye(32, dtype=np.float32)
    mixA = np.stack([np.kron(A[4 * oh:4 * oh + 4, 1:5].T, I32)
                     for oh in range(2)], axis=1)          # [128, 2, 128]
    mixB = np.stack([np.kron(A[4 * oh:4 * oh + 4, 5:8].T, I32)
                     for oh in range(2)], axis=1)          # [96, 2, 128]
    mixbias = np.stack([np.kron(A[4 * oh:4 * oh + 4, 0], np.ones(32, np.float32))
                        for oh in range(2)], axis=1)       # [128, 2]
    band = np.zeros((3, 128, 128), np.float32)  # [dw, h_in, h_out]
    for dw in range(3):
        for ho in range(128):
            for dh in range(3):
                hi = ho + dh - 1
                if 0 <= hi < 128:
                    band[dw, hi, ho] = w[dh, dw]
    return mixA, mixB, mixbias, band


def build_bass(n_cores=N_CORES, Wd=W, Cd=C, FC=512, reps=1):
    """Build the per-core Bass module. Every core runs the same program on its
    own image; collective min/max when n_cores > 1. reps>1 wraps the main
    pipeline in a For_i for wall-clock HW timing."""
    Fd = Wd * Cd
    EXT = FC + 2 * Cd
    nchunk = (Fd + FC - 1) // FC
    assert Fd % FC == 0

    from concourse.bacc import Bacc
    nc = Bacc()
    x_d = nc.declare_dram_parameter("x", [H, Fd], F32, isOutput=False)
    mixA_d = nc.declare_dram_parameter("mixA", [128, 2, 128], F32, isOutput=False)
    mixB_d = nc.declare_dram_parameter("mixB", [96, 2, 128], F32, isOutput=False)
    mixb_d = nc.declare_dram_parameter("mixb", [128, 2], F32, isOutput=False)
    band_d = nc.declare_dram_parameter("band", [3, 128, 128], F32, isOutput=False)
    y_d = nc.declare_dram_parameter("y", [H, Fd], F32, isOutput=True)
    cc_in = nc.dram_tensor("cc_in", [2], F32)
    if n_cores > 1:
        cc_out = nc.dram_tensor("cc_out", [2], F32, addr_space="Shared")

    with tile.TileContext(nc) as tc, ExitStack() as ctx:
        big = FC >= 1024  # big chunks: single-buffer the large pools to fit
        consts = ctx.enter_context(tc.tile_pool(name="consts", bufs=1))
        work = ctx.enter_context(tc.tile_pool(name="work", bufs=2))
        gpool = ctx.enter_context(tc.tile_pool(name="gpool", bufs=1 if big else 2))
        psip = ctx.enter_context(tc.tile_pool(name="psip", bufs=3))
        mixps = ctx.enter_context(tc.tile_pool(name="mixps", bufs=2, space="PSUM"))
        rpack = ctx.enter_context(tc.tile_pool(name="rpack", bufs=2 if big else 3))
        rnat = ctx.enter_context(tc.tile_pool(name="rnat", bufs=1 if big else 2))
        vpool = ctx.enter_context(tc.tile_pool(name="vpool", bufs=1 if big else 2))
        convps = ctx.enter_context(tc.tile_pool(name="convps", bufs=6, space="PSUM"))
        ypool = ctx.enter_context(tc.tile_pool(name="ypool", bufs=2 if big else 3))

        # ---- constants / inputs resident in SBUF ----
        xpad = consts.tile([128, Fd + 2 * Cd], F32)
        nc.gpsimd.memset(xpad[:, 0:Cd], 0.0)
        nc.gpsimd.memset(xpad[:, Cd + Fd:], 0.0)
        nc.sync.dma_start(out=xpad[:, Cd:Cd + Fd], in_=x_d[:, :])
        mixA = consts.tile([128, 2, 128], F32)
        nc.sync.dma_start(out=mixA, in_=mixA_d[:, :, :])
        mixB = consts.tile([96, 2, 128], F32)
        nc.sync.dma_start(out=mixB, in_=mixB_d[:, :, :])
        mixb = consts.tile([128, 2], F32)
        nc.sync.dma_start(out=mixb, in_=mixb_d[:, :])
        band = consts.tile([128, 3, 128], F32)
        nc.sync.dma_start(out=band, in_=band_d[:, :, :].rearrange("d i o -> i d o"))

        # ---- global min/max -> scale ----
        mn = consts.tile([128, 1], F32)
        mx = consts.tile([128, 1], F32)
        nc.vector.tensor_reduce(mn, xpad[:, Cd:Cd + Fd], axis=AX.X, op=OP.min)
        nc.vector.tensor_reduce(mx, xpad[:, Cd:Cd + Fd], axis=AX.X, op=OP.max)
        tmp = consts.tile([128, 2], F32)
        nc.vector.tensor_scalar_mul(tmp[:, 0:1], mn, -1.0)   # (-min, max)
        nc.vector.tensor_copy(tmp[:, 1:2], mx)
        # partition reduce on gpsimd (slow path but only 256 elements, once)
        red = consts.tile([1, 2], F32)
        nc.gpsimd.tensor_reduce(red, tmp, axis=AX.C, op=OP.max)
        pair = consts.tile([128, 2], F32)
        dma_in = nc.sync.dma_start(out=cc_in[:], in_=red)
        src = cc_in
        prev = dma_in
        if n_cores > 1:
            cc = nc.gpsimd.collective_compute(
                "AllReduce", OP.max,
                replica_groups=[list(range(n_cores))],
                ins=[cc_in.ap().opt()], outs=[cc_out.ap().opt()],
            )
            add_dep_helper(cc.ins, dma_in.ins, True, "cc waits dram write")
            src = cc_out
            prev = cc
        bcast = bass.AP(tensor=src.ap().tensor, offset=0, ap=[[0, 128], [1, 2]])
        dma_back = nc.sync.dma_start(out=pair[:, :], in_=bcast)
        # internal-DRAM round trips are not tile-tracked; order explicitly.
        add_dep_helper(dma_back.ins, prev.ins, True, "bcast waits dram ready")
        negxmin = pair[:, 0:1]
        gmax = pair[:, 1:2]

        rng = consts.tile([128, 1], F32)
        nc.vector.tensor_tensor(rng, gmax, negxmin, op=OP.add)   # xmax - xmin
        dd = consts.tile([128, 1], F32)
        nc.vector.tensor_scalar_add(dd, rng, float(np.float32(1e-8)))
        # scale = 8/d as 8*(1/d): exact wrt fl(8/d) since *8 is a pow2 scale
        recip = consts.tile([128, 1], F32)
        nc.vector.reciprocal(recip, dd)
        scale = consts.tile([128, 1], F32)
        nc.vector.tensor_scalar_mul(scale, recip, 8.0)

        # ---- main streamed pipeline over free-dim chunks ----
        def chunk_pipeline(ci):
            cs = ci * FC  # xpad col cs .. cs+EXT covers data cols [cs-Cd, cs+FC+Cd)
            u = work.tile([128, EXT], F32, tag="u")
            nc.vector.tensor_scalar(u, xpad[:, cs:cs + EXT], negxmin, scale,
                                    op0=OP.add, op1=OP.mult)
            g = gpool.tile([128, 7, EXT], F32, tag="g")
            for j in range(7):
                nc.vector.tensor_scalar(g[:, j, :], u, float(j + 1), None,
                                        op0=OP.is_ge)

            # pack staircases per 32-row h-group: psiA[(j-1,hl32),:], psiB[(j-5,hl32),:]
            rho = rnat.tile([128, NB, EXT], F32, tag="rho")
            for grp in range(4):
                psiA = psip.tile([128, EXT], F32, tag="psiA")
                psiB = psip.tile([96, EXT], F32, tag="psiB")
                for j in range(1, 5):
                    nc.sync.dma_start(out=psiA[32 * (j - 1):32 * j, :],
                                      in_=g[32 * grp:32 * grp + 32, j - 1, :])
                for j in range(5, 8):
                    nc.sync.dma_start(out=psiB[32 * (j - 5):32 * (j - 4), :],
                                      in_=g[32 * grp:32 * grp + 32, j - 1, :])
                for oh in range(2):
                    rp = rpack.tile([128, EXT], F32, tag="rp")
                    for s in range(0, EXT, 512):
                        e = min(EXT, s + 512)
                        # one PSUM bank per span
                        pm = mixps.tile([128, e - s], F32, tag="pm")
                        nc.tensor.matmul(pm, mixA[:, oh, :], psiA[:, s:e],
                                         start=True, stop=False)
                        nc.tensor.matmul(pm, mixB[:, oh, :], psiB[:, s:e],
                                         start=False, stop=True)
                        # evacuate PSUM on ACT (otherwise idle); fold the g_0
                        # (ones) column via the per-partition bias
                        nc.scalar.activation(rp[:, s:e], pm,
                                             mybir.ActivationFunctionType.Identity,
                                             bias=mixb[:, oh:oh + 1], scale=1.0)
                    # unpack rows (il,hl32) -> rho[grp*32+hl, 4*oh+il, :]
                    # (HWDGE via the scalar queue; gpsimd SWDGE is slow)
                    for il in range(4):
                        nc.scalar.dma_start(
                            out=rho[32 * grp:32 * grp + 32, 4 * oh + il, :],
                            in_=rp[32 * il:32 * il + 32, :])

            v = vpool.tile([128, NB, EXT], F32, tag="v")
            for i in range(NB):
                nc.vector.tensor_tensor(v[:, i, :], xpad[:, cs:cs + EXT],
                                        rho[:, i, :], op=OP.mult)

            y_t = ypool.tile([128, FC], F32, tag="y")
            for sub in range(0, FC, 512):
                sw = min(512, FC - sub)  # conv output sub-chunk (<=1 PSUM bank)
                for half in range(2):
                    vts = []
                    for i in range(4 * half, 4 * half + 4):
                        vt = convps.tile([128, sw], F32, tag="vt")
                        vts.append(vt)
                    for dw in range(3):
                        for k, i in enumerate(range(4 * half, 4 * half + 4)):
                            nc.tensor.matmul(
                                vts[k], band[:, dw, :],
                                v[:, i, sub + dw * Cd:sub + dw * Cd + sw],
                                start=(dw == 0), stop=(dw == 2))
                    for k, i in enumerate(range(4 * half, 4 * half + 4)):
                        if i == 0:
                            nc.vector.tensor_copy(y_t[:, sub:sub + sw], vts[k])
                        else:
                            # overwrite where g_i != 0 (ascending i => y=V_idx);
                            # mask must be int-typed: bitcast the 0.0/1.0 f32
                            mask = g[:, i - 1, Cd + sub:Cd + sub + sw].bitcast(
                                mybir.dt.uint32)
                            nc.vector.copy_predicated(y_t[:, sub:sub + sw],
                                                      mask, vts[k])
            nc.sync.dma_start(out=y_d[:, cs:cs + FC], in_=y_t)

        if reps == 1:
            for ci in range(nchunk):
                chunk_pipeline(ci)
        else:
            with tc.For_i(0, reps, 1):
                for ci in range(nchunk):
                    chunk_pipeline(ci)
    nc.finalize()  # runs the Bacc compile pipeline (reg alloc, wait splitting)
    return nc


_CACHE = {}


def _run(x, co_matrix, w_spatial, trace=False):
    x = np.ascontiguousarray(np.asarray(x, np.float32))
    mixA, mixB, mixb, band = build_tables(co_matrix, w_spatial)
    n, h, w_, c = x.shape
    assert (n, h, w_, c) == (N, H, W, C), (n, h, w_, c)

    from concourse.bass_utils import run_bass_kernel_spmd

    key = "full"
    if key not in _CACHE:
        _CACHE[key] = build_bass(n_cores=N_CORES, FC=1024)
    nc = _CACHE[key]

    in_maps = []
    for core in range(N_CORES):
        in_maps.append({
            "x": x[core].reshape(H, W * C),
            "mixA": mixA,
            "mixB": mixB,
            "mixb": mixb,
            "band": band,
        })
    res = run_bass_kernel_spmd(nc, in_maps, core_ids=list(range(N_CORES)),
                               trace=trace)
    out = np.stack([res.results[i]["y"].reshape(H, W, C) for i in range(N_CORES)], 0)
    return out, res


def kernel(x, co_matrix, w_spatial):
    return _run(x, co_matrix, w_spatial)[0]


def run_traced(x, co_matrix, w_spatial):
    _, res = _run(x, co_matrix, w_spatial, trace=True)
    return res.exec_time_ns

